# revision 4
# baseline (speedup 1.0000x reference)
"""Multi-head causal attention (B=4, S=2048, D=1024, H=16, HD=64) on 8 TRN2 cores.

Strategy:
  - Head-parallel: core i computes heads {2i, 2i+1} for all tokens.
    Host pre-transposes x -> xT [D, B*S] and folds the 1/sqrt(HD) scale into Wq.
  - On device per core: qT/kT/vT projections (weights stationary, xT moving),
    scores computed transposed [k, q] with the two heads packed via PE row
    tiling (K=64 each), exp on ACT, PV matmul with stationary [v | 1] so the
    softmax denominator lands in output row 64, normalize via reciprocal +
    partition_broadcast.
  - One AllToAll reshards head-outputs (feature-major) to token-slices; each
    core then runs the output projection for its 1024 tokens. Host concatenates
    core outputs and adds bo.
"""

import os
import sys

sys.path.insert(0, "/opt/trn_rl_repo")

import numpy as np

import concourse.bass as bass
import concourse.mybir as mybir
import concourse.tile as tile
from concourse import bacc, bass_utils

FP = mybir.dt.float32
AOP = mybir.AluOpType
AFT = mybir.ActivationFunctionType

B, S, D, H = 4, 2048, 1024, 16
HD = 64
N_CORES = 8
NT = B * S  # 8192 tokens
TOK_PER_CORE = NT // N_CORES  # 1024
KD = D // 128  # 8 contraction tiles for the projections


def build_nc():
    nc = bacc.Bacc(None, target_bir_lowering=False, debug=False, num_devices=N_CORES)

    xt = nc.dram_tensor("xt", [D, NT], FP, kind="ExternalInput")
    wqk = nc.dram_tensor("wqk", [2, D, 128], FP, kind="ExternalInput")
    wv = nc.dram_tensor("wv", [D, 128], FP, kind="ExternalInput")
    bqk = nc.dram_tensor("bqk", [2, 128, 1], FP, kind="ExternalInput")
    bvb = nc.dram_tensor("bv", [128, 1], FP, kind="ExternalInput")
    wo = nc.dram_tensor("wo", [D, D], FP, kind="ExternalInput")
    maskd = nc.dram_tensor("mask", [128, 896], FP, kind="ExternalInput")
    identd = nc.dram_tensor("ident", [128, 128], FP, kind="ExternalInput")
    out = nc.dram_tensor("out", [TOK_PER_CORE, D], FP, kind="ExternalOutput")

    with tile.TileContext(nc) as tc:
        with (
            tc.tile_pool(name="const", bufs=1) as const,
            tc.tile_pool(name="xtp", bufs=12) as xtp,
            tc.tile_pool(name="qkv", bufs=2) as qkv,
            tc.tile_pool(name="vnp", bufs=18) as vnp,
            tc.tile_pool(name="esp", bufs=3) as esp,
            tc.tile_pool(name="small", bufs=4) as small,
            tc.tile_pool(name="onp", bufs=4) as onp,
            tc.tile_pool(name="actp", bufs=10) as actp,
            tc.tile_pool(name="oop", bufs=3) as oop,
            tc.tile_pool(name="ps_mm", bufs=2, space="PSUM") as ps_mm,
            tc.tile_pool(name="ps_s", bufs=2, space="PSUM") as ps_s,
            tc.tile_pool(name="ps_o", bufs=1, space="PSUM") as ps_o,
            tc.tile_pool(name="dram", bufs=1, space="DRAM") as dram,
        ):
            cc_in = dram.tile([N_CORES, 128, TOK_PER_CORE], FP, name="cc_in")
            cc_out = dram.tile([N_CORES, 128, TOK_PER_CORE], FP, name="cc_out")

            # ---- resident constants ----
            mask_sb = const.tile([128, 896], FP, name="mask_sb")
            nc.sync.dma_start(mask_sb[:], maskd[:])
            ident_sb = const.tile([128, 128], FP, name="ident_sb")
            nc.sync.dma_start(ident_sb[:], identd[:])
            wqk_sb = const.tile([128, 2 * KD, 128], FP, name="wqk_sb")
            nc.sync.dma_start(
                wqk_sb[:],
                wqk.rearrange("h (ko p) m -> p (h ko) m", p=128),
            )
            wv_sb = const.tile([128, KD, 128], FP, name="wv_sb")
            nc.sync.dma_start(wv_sb[:], wv.rearrange("(ko p) m -> p ko m", p=128))
            bqk_sb = const.tile([128, 2], FP, name="bqk_sb")
            nc.sync.dma_start(bqk_sb[:], bqk.rearrange("h p one -> p (h one)"))
            bv_sb = const.tile([128, 1], FP, name="bv_sb")
            nc.sync.dma_start(bv_sb[:], bvb[:])
            wo_sb = const.tile([128, KD, D], FP, name="wo_sb")
            nc.sync.dma_start(wo_sb[:], wo.rearrange("(ko p) n -> p ko n", p=128))

            for b in range(B):
                # ---- QKV projection for this batch (both heads) ----
                qT = qkv.tile([128, S], FP, name="qT", tag="qT")  # rows: h0 dims 0-63, h1 64-127
                kT = qkv.tile([128, S], FP, name="kT", tag="kT")
                vT = qkv.tile([128, S], FP, name="vT", tag="vT")
                for st in range(4):  # 512-token slabs
                    xts = []
                    for kd in range(KD):
                        xt_t = xtp.tile([128, 512], FP, name="xt_t", tag="xt")
                        nc.sync.dma_start(
                            xt_t[:],
                            xt[kd * 128 : (kd + 1) * 128, b * S + st * 512 : b * S + (st + 1) * 512],
                        )
                        xts.append(xt_t)
                    for h in range(2):
                        ps = ps_mm.tile([128, 512], FP, name="ps_qk", tag="mm")
                        for kd in range(KD):
                            nc.tensor.matmul(
                                ps[:],
                                lhsT=wqk_sb[:, h * KD + kd, :],
                                rhs=xts[kd][:],
                                start=(kd == 0),
                                stop=(kd == KD - 1),
                            )
                        nc.vector.tensor_scalar(
                            qT[h * 64 : h * 64 + 64, st * 512 : (st + 1) * 512],
                            ps[0:64, :],
                            bqk_sb[0:64, h : h + 1],
                            None,
                            AOP.add,
                        )
                        nc.vector.tensor_scalar(
                            kT[h * 64 : h * 64 + 64, st * 512 : (st + 1) * 512],
                            ps[64:128, :],
                            bqk_sb[64:128, h : h + 1],
                            None,
                            AOP.add,
                        )
                    ps = ps_mm.tile([128, 512], FP, name="ps_v", tag="mm")
                    for kd in range(KD):
                        nc.tensor.matmul(
                            ps[:],
                            lhsT=wv_sb[:, kd, :],
                            rhs=xts[kd][:],
                            start=(kd == 0),
                            stop=(kd == KD - 1),
                        )
                    nc.vector.tensor_scalar(
                        vT[:, st * 512 : (st + 1) * 512],
                        ps[:],
                        bv_sb[:, 0:1],
                        None,
                        AOP.add,
                    )

                # ---- vT -> v natural [token, hd] with ones columns ----
                vn_tiles = []
                for kc in range(S // 128):
                    pst = ps_mm.tile([128, 128], FP, name="ps_t", tag="mm")
                    nc.tensor.transpose(pst[:], vT[:, kc * 128 : (kc + 1) * 128], ident_sb[:])
                    vn = vnp.tile([128, 130], FP, name="vn", tag="vn")
                    nc.any.tensor_copy(out=vn[:, 0:64], in_=pst[:, 0:64])
                    nc.any.tensor_copy(out=vn[:, 65:129], in_=pst[:, 64:128])
                    nc.vector.memset(vn[:, 64:65], 1.0)
                    nc.vector.memset(vn[:, 129:130], 1.0)
                    vn_tiles.append(vn)

                # ---- causal attention, scores transposed [k, q] ----
                for qi in range(4):  # 512-wide query tiles
                    po = [
                        ps_o.tile([65, 512], FP, name=f"po{h}", tag=f"o{h}")
                        for h in range(2)
                    ]
                    nki = 4 * (qi + 1)
                    for ki in range(nki):
                        pss = ps_s.tile([128, 1024], FP, name="ps_sc", tag="sc")
                        for h in range(2):
                            nc.tensor.matmul(
                                pss[:, h * 512 : (h + 1) * 512],
                                lhsT=kT[h * 64 : h * 64 + 64, ki * 128 : (ki + 1) * 128],
                                rhs=qT[h * 64 : h * 64 + 64, qi * 512 : (qi + 1) * 512],
                                start=True,
                                stop=True,
                                tile_position=(h * 64, 0),
                            )
                        es = esp.tile([128, 1024], FP, name="es", tag="es")
                        nc.scalar.activation(es[:], pss[:], AFT.Exp)
                        if ki >= 4 * qi:  # diagonal tile: multiplicative causal mask
                            j = ki - 4 * qi
                            for h in range(2):
                                nc.vector.tensor_tensor(
                                    es[:, h * 512 : (h + 1) * 512],
                                    es[:, h * 512 : (h + 1) * 512],
                                    mask_sb[:, 384 - 128 * j : 896 - 128 * j],
                                    AOP.mult,
                                )
                        for h in range(2):
                            nc.tensor.matmul(
                                po[h][:],
                                lhsT=vn_tiles[ki][:, h * 65 : (h + 1) * 65],
                                rhs=es[:, h * 512 : (h + 1) * 512],
                                start=(ki == 0),
                                stop=(ki == nki - 1),
                            )
                    # normalize and ship to the A2A send buffer
                    j_rank = 2 * b + qi // 2
                    off = (qi % 2) * 512
                    for h in range(2):
                        rec = small.tile([1, 512], FP, name="rec", tag="rec")
                        nc.vector.reciprocal(rec[:], po[h][64:65, :])
                        bc = small.tile([64, 512], FP, name="bc", tag="bc")
                        nc.gpsimd.partition_broadcast(bc[:], rec[:], channels=64)
                        on = onp.tile([64, 512], FP, name="on", tag="on")
                        nc.vector.tensor_tensor(on[:], po[h][0:64, :], bc[:], AOP.mult)
                        nc.sync.dma_start(
                            cc_in[j_rank, h * 64 : (h + 1) * 64, off : off + 512],
                            on[:],
                        )

            # ---- reshard heads -> tokens ----
            nc.gpsimd.collective_compute(
                "AllToAll",
                AOP.bypass,
                replica_groups=[list(range(N_CORES))],
                ins=[cc_in[:].opt()],
                outs=[cc_out[:].opt()],
            )

            # ---- output projection for my 1024 tokens ----
            for tt in range(TOK_PER_CORE // 128):
                acts = []
                for ft in range(N_CORES):
                    at = actp.tile([128, 128], FP, name="at", tag="at")
                    nc.sync.dma_start(at[:], cc_out[ft, :, tt * 128 : (tt + 1) * 128])
                    acts.append(at)
                for nn in range(2):
                    ps = ps_mm.tile([128, 512], FP, name="ps_op", tag="mm")
                    for ft in range(N_CORES):
                        nc.tensor.matmul(
                            ps[:],
                            lhsT=acts[ft][:],
                            rhs=wo_sb[:, ft, nn * 512 : (nn + 1) * 512],
                            start=(ft == 0),
                            stop=(ft == N_CORES - 1),
                        )
                    oo = oop.tile([128, 512], FP, name="oo", tag="oo")
                    nc.any.tensor_copy(out=oo[:], in_=ps[:])
                    nc.sync.dma_start(
                        out[tt * 128 : (tt + 1) * 128, nn * 512 : (nn + 1) * 512], oo[:]
                    )

    nc.finalize()
    return nc


_NC_CACHE = None


def _get_nc():
    global _NC_CACHE
    if _NC_CACHE is None:
        _NC_CACHE = build_nc()
    return _NC_CACHE


def make_in_maps(x, Wqkv, bqkv, Wo):
    scale = HD ** -0.5
    xtn = np.ascontiguousarray(x.reshape(NT, D).T)  # [D, NT]
    mask = (np.arange(896)[None, :] - 384 >= np.arange(128)[:, None]).astype(np.float32)
    ident = np.eye(128, dtype=np.float32)
    wo = np.ascontiguousarray(Wo)
    in_maps = []
    for c in range(N_CORES):
        h0, h1 = 2 * c, 2 * c + 1
        wqk_c = np.stack(
            [
                np.concatenate(
                    [Wqkv[h][:, 0:64] * scale, Wqkv[h][:, 64:128]], axis=1
                )
                for h in (h0, h1)
            ]
        ).astype(np.float32)
        wv_c = np.concatenate(
            [Wqkv[h0][:, 128:192], Wqkv[h1][:, 128:192]], axis=1
        ).astype(np.float32)
        bqk_c = np.stack(
            [
                np.concatenate([bqkv[h][0:64] * scale, bqkv[h][64:128]])[:, None]
                for h in (h0, h1)
            ]
        ).astype(np.float32)
        bv_c = np.concatenate([bqkv[h0][128:192], bqkv[h1][128:192]])[:, None].astype(
            np.float32
        )
        in_maps.append(
            {
                "xt": xtn,
                "wqk": np.ascontiguousarray(wqk_c),
                "wv": np.ascontiguousarray(wv_c),
                "bqk": np.ascontiguousarray(bqk_c),
                "bv": np.ascontiguousarray(bv_c),
                "wo": wo,
                "mask": mask,
                "ident": ident,
            }
        )
    return in_maps


def run_cores(in_maps, trace=False, trace_kwargs=None):
    nc = _get_nc()
    kwargs = {}
    if trace:
        kwargs["trace"] = True
        if trace_kwargs:
            kwargs["trace_kwargs"] = trace_kwargs
    return bass_utils.run_bass_kernel_spmd(
        nc, in_maps, core_ids=list(range(N_CORES)), **kwargs
    )


def kernel(x, Wqkv, bqkv, Wo, bo):
    x = np.asarray(x, dtype=np.float32)
    Wqkv = np.asarray(Wqkv, dtype=np.float32)
    bqkv = np.asarray(bqkv, dtype=np.float32)
    Wo = np.asarray(Wo, dtype=np.float32)
    bo = np.asarray(bo, dtype=np.float32)

    in_maps = make_in_maps(x, Wqkv, bqkv, Wo)
    res = run_cores(in_maps)
    full = np.concatenate([res.results[c]["out"] for c in range(N_CORES)], axis=0)
    full = full + bo[None, :]
    return full.reshape(B, S, D)


# revision 11
# speedup vs baseline: 2.0665x; 2.0665x over previous
"""Multi-head causal attention (B=4, S=2048, D=1024, H=16, HD=64) on 8 TRN2 cores.

Strategy:
  - Head-parallel: core i computes heads {2i, 2i+1} for all tokens.
    Host pre-transposes x -> xT [D, B*S] and folds the 1/sqrt(HD) scale into Wq.
  - On device per core: qT/kT/vT projections (weights stationary, xT moving),
    scores computed transposed [k, q] with the two heads packed via PE row
    tiling (K=64 each), exp on ACT, PV matmul with stationary [v | 1] so the
    softmax denominator lands in output row 64, normalize via reciprocal +
    partition_broadcast.
  - One AllToAll reshards head-outputs (feature-major) to token-slices; each
    core then runs the output projection for its 1024 tokens. Host concatenates
    core outputs and adds bo.
"""

import os
import sys

sys.path.insert(0, "/opt/trn_rl_repo")

import numpy as np

import concourse.bass as bass
import concourse.mybir as mybir
import concourse.tile as tile
from concourse import bacc, bass_utils

FP = mybir.dt.float32
AOP = mybir.AluOpType
AFT = mybir.ActivationFunctionType
FPR = mybir.dt.float32r


def _r(ap):
    """View an fp32 AP as float32r for full-rate PE matmuls."""
    return ap.bitcast(FPR)

B, S, D, H = 4, 2048, 1024, 16
HD = 64
N_CORES = 8
NT = B * S  # 8192 tokens
TOK_PER_CORE = NT // N_CORES  # 1024
KD = D // 128  # 8 contraction tiles for the projections


def build_nc():
    nc = bacc.Bacc(None, target_bir_lowering=False, debug=False, num_devices=N_CORES)

    xt = nc.dram_tensor("xt", [D, NT], FP, kind="ExternalInput")
    wqk = nc.dram_tensor("wqk", [2, D, 128], FP, kind="ExternalInput")
    wv = nc.dram_tensor("wv", [D, 128], FP, kind="ExternalInput")
    bqk = nc.dram_tensor("bqk", [2, 128, 1], FP, kind="ExternalInput")
    bvb = nc.dram_tensor("bv", [128, 1], FP, kind="ExternalInput")
    wo = nc.dram_tensor("wo", [D, D], FP, kind="ExternalInput")
    maskd = nc.dram_tensor("mask", [128, 896], FP, kind="ExternalInput")
    identd = nc.dram_tensor("ident", [128, 128], FP, kind="ExternalInput")
    out = nc.dram_tensor("out", [TOK_PER_CORE, D], FP, kind="ExternalOutput")

    with tile.TileContext(nc) as tc:
        with (
            tc.tile_pool(name="const", bufs=1) as const,
            tc.tile_pool(name="xtp", bufs=12) as xtp,
            tc.tile_pool(name="qkv", bufs=2) as qkv,
            tc.tile_pool(name="vnp", bufs=18) as vnp,
            tc.tile_pool(name="esp", bufs=3) as esp,
            tc.tile_pool(name="small", bufs=4) as small,
            tc.tile_pool(name="onp", bufs=4) as onp,
            tc.tile_pool(name="actp", bufs=10) as actp,
            tc.tile_pool(name="oop", bufs=3) as oop,
            tc.tile_pool(name="ps_mm", bufs=2, space="PSUM") as ps_mm,
            tc.tile_pool(name="ps_s", bufs=2, space="PSUM") as ps_s,
            tc.tile_pool(name="ps_o", bufs=1, space="PSUM") as ps_o,
            tc.tile_pool(name="dram", bufs=1, space="DRAM") as dram,
        ):
            cc_in = dram.tile([N_CORES, 128, TOK_PER_CORE], FP, name="cc_in")
            cc_out = dram.tile([N_CORES, 128, TOK_PER_CORE], FP, name="cc_out")

            # ---- resident constants ----
            mask_sb = const.tile([128, 896], FPR, name="mask_sb")
            nc.sync.dma_start(mask_sb[:], maskd[:].bitcast(FPR))
            ident_sb = const.tile([128, 128], FPR, name="ident_sb")
            nc.sync.dma_start(ident_sb[:], identd[:].bitcast(FPR))
            wqk_sb = const.tile([128, 2 * KD, 128], FPR, name="wqk_sb")
            nc.sync.dma_start(
                wqk_sb[:],
                wqk.rearrange("h (ko p) m -> p (h ko) m", p=128).bitcast(FPR),
            )
            wv_sb = const.tile([128, KD, 128], FPR, name="wv_sb")
            nc.sync.dma_start(wv_sb[:], wv.rearrange("(ko p) m -> p ko m", p=128).bitcast(FPR))
            bqk_sb = const.tile([128, 2], FP, name="bqk_sb")
            nc.sync.dma_start(bqk_sb[:], bqk.rearrange("h p one -> p (h one)"))
            bv_sb = const.tile([128, 1], FP, name="bv_sb")
            nc.sync.dma_start(bv_sb[:], bvb[:])
            for b in range(B):
                # ---- QKV projection for this batch (both heads) ----
                qT = qkv.tile([128, S], FPR, name="qT", tag="qT")  # rows: h0 dims 0-63, h1 64-127
                kT = qkv.tile([128, S], FPR, name="kT", tag="kT")
                vT = qkv.tile([128, S], FPR, name="vT", tag="vT")
                for st in range(4):  # 512-token slabs
                    xts = []
                    for kd in range(KD):
                        xt_t = xtp.tile([128, 512], FPR, name="xt_t", tag="xt")
                        nc.sync.dma_start(
                            xt_t[:],
                            xt[kd * 128 : (kd + 1) * 128, b * S + st * 512 : b * S + (st + 1) * 512].bitcast(FPR),
                        )
                        xts.append(xt_t)
                    for h in range(2):
                        ps = ps_mm.tile([128, 512], FP, name="ps_qk", tag="mm")
                        for kd in range(KD):
                            nc.tensor.matmul(
                                ps[:],
                                lhsT=wqk_sb[:, h * KD + kd, :],
                                rhs=xts[kd][:],
                                start=(kd == 0),
                                stop=(kd == KD - 1),
                            )
                        nc.vector.tensor_scalar(
                            qT[h * 64 : h * 64 + 64, st * 512 : (st + 1) * 512],
                            ps[0:64, :],
                            bqk_sb[0:64, h : h + 1],
                            None,
                            AOP.add,
                        )
                        nc.vector.tensor_scalar(
                            kT[h * 64 : h * 64 + 64, st * 512 : (st + 1) * 512],
                            ps[64:128, :],
                            bqk_sb[64:128, h : h + 1],
                            None,
                            AOP.add,
                        )
                    ps = ps_mm.tile([128, 512], FP, name="ps_v", tag="mm")
                    for kd in range(KD):
                        nc.tensor.matmul(
                            ps[:],
                            lhsT=wv_sb[:, kd, :],
                            rhs=xts[kd][:],
                            start=(kd == 0),
                            stop=(kd == KD - 1),
                        )
                    nc.vector.tensor_scalar(
                        vT[:, st * 512 : (st + 1) * 512],
                        ps[:],
                        bv_sb[:, 0:1],
                        None,
                        AOP.add,
                    )

                # ---- vT -> v natural [token, hd] with ones columns ----
                vn_tiles = []
                for kc in range(S // 128):
                    pst = ps_mm.tile([128, 128], FPR, name="ps_t", tag="mm")
                    nc.tensor.transpose(pst[:], vT[:, kc * 128 : (kc + 1) * 128], ident_sb[:])
                    vn = vnp.tile([128, 130], FPR, name="vn", tag="vn")
                    nc.any.tensor_copy(out=vn[:, 0:64], in_=pst[:, 0:64])
                    nc.any.tensor_copy(out=vn[:, 65:129], in_=pst[:, 64:128])
                    nc.vector.tensor_copy(out=vn[:, 64:65], in_=mask_sb[:, 895:896])
                    nc.vector.tensor_copy(out=vn[:, 129:130], in_=mask_sb[:, 895:896])
                    vn_tiles.append(vn)

                # ---- causal attention, scores transposed [k, q] ----
                for qi in range(4):  # 512-wide query tiles
                    po = [
                        ps_o.tile([65, 512], FP, name=f"po{h}", tag=f"o{h}")
                        for h in range(2)
                    ]
                    nki = 4 * (qi + 1)
                    for ki in range(nki):
                        pss = ps_s.tile([128, 1024], FP, name="ps_sc", tag="sc")
                        for h in range(2):
                            nc.tensor.matmul(
                                pss[:, h * 512 : (h + 1) * 512],
                                lhsT=kT[h * 64 : h * 64 + 64, ki * 128 : (ki + 1) * 128],
                                rhs=qT[h * 64 : h * 64 + 64, qi * 512 : (qi + 1) * 512],
                                start=True,
                                stop=True,
                                tile_position=(h * 64, 0),
                            )
                        es = esp.tile([128, 1024], FPR, name="es", tag="es")
                        nc.scalar.activation(es[:], pss[:], AFT.Exp)
                        if ki >= 4 * qi:  # diagonal tile: multiplicative causal mask
                            j = ki - 4 * qi
                            for h in range(2):
                                nc.vector.tensor_tensor(
                                    es[:, h * 512 : (h + 1) * 512],
                                    es[:, h * 512 : (h + 1) * 512],
                                    mask_sb[:, 384 - 128 * j : 896 - 128 * j],
                                    AOP.mult,
                                )
                        for h in range(2):
                            nc.tensor.matmul(
                                po[h][:],
                                lhsT=vn_tiles[ki][:, h * 65 : (h + 1) * 65],
                                rhs=es[:, h * 512 : (h + 1) * 512],
                                start=(ki == 0),
                                stop=(ki == nki - 1),
                            )
                    # normalize and ship to the A2A send buffer
                    j_rank = 2 * b + qi // 2
                    off = (qi % 2) * 512
                    for h in range(2):
                        oc = small.tile([65, 512], FP, name="oc", tag="oc")
                        nc.vector.tensor_copy(out=oc[:], in_=po[h][:])
                        den = small.tile([1, 512], FP, name="den", tag="den")
                        nc.vector.tensor_copy(out=den[:], in_=oc[64:65, :])
                        bc = small.tile([64, 512], FP, name="bc", tag="bc")
                        nc.gpsimd.partition_broadcast(bc[:], den[0:1, :], channels=64)
                        rc = small.tile([64, 512], FP, name="rc", tag="rc")
                        nc.vector.reciprocal(rc[:], bc[:])
                        on = onp.tile([64, 512], FP, name="on", tag="on")
                        nc.vector.tensor_tensor(on[:], oc[0:64, :], rc[:], AOP.mult)
                        nc.sync.dma_start(
                            cc_in[j_rank, h * 64 : (h + 1) * 64, off : off + 512],
                            on[:],
                        )

            wo_sb = const.tile([128, KD, D], FPR, name="wo_sb")
            nc.sync.dma_start(wo_sb[:], wo.rearrange("(ko p) n -> p ko n", p=128).bitcast(FPR))

            # ---- reshard heads -> tokens ----
            nc.gpsimd.collective_compute(
                "AllToAll",
                AOP.bypass,
                replica_groups=[list(range(N_CORES))],
                ins=[cc_in[:].opt()],
                outs=[cc_out[:].opt()],
            )

            # ---- output projection for my 1024 tokens ----
            for tt in range(TOK_PER_CORE // 128):
                acts = []
                for ft in range(N_CORES):
                    at = actp.tile([128, 128], FPR, name="at", tag="at")
                    nc.sync.dma_start(at[:], cc_out[ft, :, tt * 128 : (tt + 1) * 128].bitcast(FPR))
                    acts.append(at)
                for nn in range(2):
                    ps = ps_mm.tile([128, 512], FP, name="ps_op", tag="mm")
                    for ft in range(N_CORES):
                        nc.tensor.matmul(
                            ps[:],
                            lhsT=acts[ft][:],
                            rhs=wo_sb[:, ft, nn * 512 : (nn + 1) * 512],
                            start=(ft == 0),
                            stop=(ft == N_CORES - 1),
                        )
                    oo = oop.tile([128, 512], FP, name="oo", tag="oo")
                    nc.any.tensor_copy(out=oo[:], in_=ps[:])
                    nc.sync.dma_start(
                        out[tt * 128 : (tt + 1) * 128, nn * 512 : (nn + 1) * 512], oo[:]
                    )

    nc.finalize()
    return nc


_NC_CACHE = None


def _get_nc():
    global _NC_CACHE
    if _NC_CACHE is None:
        _NC_CACHE = build_nc()
    return _NC_CACHE


def make_in_maps(x, Wqkv, bqkv, Wo):
    scale = HD ** -0.5
    xtn = np.ascontiguousarray(x.reshape(NT, D).T)  # [D, NT]
    mask = (np.arange(896)[None, :] - 384 >= np.arange(128)[:, None]).astype(np.float32)
    ident = np.eye(128, dtype=np.float32)
    wo = np.ascontiguousarray(Wo)
    in_maps = []
    for c in range(N_CORES):
        h0, h1 = 2 * c, 2 * c + 1
        wqk_c = np.stack(
            [
                np.concatenate(
                    [Wqkv[h][:, 0:64] * scale, Wqkv[h][:, 64:128]], axis=1
                )
                for h in (h0, h1)
            ]
        ).astype(np.float32)
        wv_c = np.concatenate(
            [Wqkv[h0][:, 128:192], Wqkv[h1][:, 128:192]], axis=1
        ).astype(np.float32)
        bqk_c = np.stack(
            [
                np.concatenate([bqkv[h][0:64] * scale, bqkv[h][64:128]])[:, None]
                for h in (h0, h1)
            ]
        ).astype(np.float32)
        bv_c = np.concatenate([bqkv[h0][128:192], bqkv[h1][128:192]])[:, None].astype(
            np.float32
        )
        in_maps.append(
            {
                "xt": xtn,
                "wqk": np.ascontiguousarray(wqk_c),
                "wv": np.ascontiguousarray(wv_c),
                "bqk": np.ascontiguousarray(bqk_c),
                "bv": np.ascontiguousarray(bv_c),
                "wo": wo,
                "mask": mask,
                "ident": ident,
            }
        )
    return in_maps


def run_cores(in_maps, trace=False, trace_kwargs=None):
    nc = _get_nc()
    kwargs = {}
    if trace:
        kwargs["trace"] = True
        if trace_kwargs:
            kwargs["trace_kwargs"] = trace_kwargs
    return bass_utils.run_bass_kernel_spmd(
        nc, in_maps, core_ids=list(range(N_CORES)), **kwargs
    )


def kernel(x, Wqkv, bqkv, Wo, bo):
    x = np.asarray(x, dtype=np.float32)
    Wqkv = np.asarray(Wqkv, dtype=np.float32)
    bqkv = np.asarray(bqkv, dtype=np.float32)
    Wo = np.asarray(Wo, dtype=np.float32)
    bo = np.asarray(bo, dtype=np.float32)

    in_maps = make_in_maps(x, Wqkv, bqkv, Wo)
    res = run_cores(in_maps)
    full = np.concatenate([res.results[c]["out"] for c in range(N_CORES)], axis=0)
    full = full + bo[None, :]
    return full.reshape(B, S, D)


# revision 12
# speedup vs baseline: 2.8619x; 1.3849x over previous
"""Multi-head causal attention (B=4, S=2048, D=1024, H=16, HD=64) on 8 TRN2 cores.

Strategy:
  - Head-parallel: core i computes heads {2i, 2i+1} for all tokens.
    Host pre-transposes x -> xT [D, B*S] and folds the 1/sqrt(HD) scale into Wq.
  - On device per core: qT/kT/vT projections (weights stationary, xT moving),
    scores computed transposed [k, q] with the two heads packed via PE row
    tiling (K=64 each), exp on ACT, PV matmul with stationary [v | 1] so the
    softmax denominator lands in output row 64, normalize via reciprocal +
    partition_broadcast.
  - One AllToAll reshards head-outputs (feature-major) to token-slices; each
    core then runs the output projection for its 1024 tokens. Host concatenates
    core outputs and adds bo.
"""

import os
import sys

sys.path.insert(0, "/opt/trn_rl_repo")

import numpy as np

import concourse.bass as bass
import concourse.mybir as mybir
import concourse.tile as tile
from concourse import bacc, bass_utils

FP = mybir.dt.float32
AOP = mybir.AluOpType
AFT = mybir.ActivationFunctionType
FPR = mybir.dt.float32r
BF = mybir.dt.bfloat16


def _r(ap):
    """View an fp32 AP as float32r for full-rate PE matmuls."""
    return ap.bitcast(FPR)

B, S, D, H = 4, 2048, 1024, 16
HD = 64
N_CORES = 8
NT = B * S  # 8192 tokens
TOK_PER_CORE = NT // N_CORES  # 1024
KD = D // 128  # 8 contraction tiles for the projections


def build_nc():
    nc = bacc.Bacc(None, target_bir_lowering=False, debug=False, num_devices=N_CORES)

    xt = nc.dram_tensor("xt", [D, NT], BF, kind="ExternalInput")
    wqk = nc.dram_tensor("wqk", [2, D, 128], BF, kind="ExternalInput")
    wv = nc.dram_tensor("wv", [D, 128], BF, kind="ExternalInput")
    bqk = nc.dram_tensor("bqk", [2, 128, 1], FP, kind="ExternalInput")
    bvb = nc.dram_tensor("bv", [128, 1], FP, kind="ExternalInput")
    wo = nc.dram_tensor("wo", [D, D], BF, kind="ExternalInput")
    maskd = nc.dram_tensor("mask", [128, 896], BF, kind="ExternalInput")
    identd = nc.dram_tensor("ident", [128, 128], BF, kind="ExternalInput")
    out = nc.dram_tensor("out", [TOK_PER_CORE, D], FP, kind="ExternalOutput")

    with tile.TileContext(nc) as tc:
        with (
            tc.tile_pool(name="const", bufs=1) as const,
            tc.tile_pool(name="xtp", bufs=12) as xtp,
            tc.tile_pool(name="qkv", bufs=2) as qkv,
            tc.tile_pool(name="vnp", bufs=18) as vnp,
            tc.tile_pool(name="esp", bufs=3) as esp,
            tc.tile_pool(name="small", bufs=4) as small,
            tc.tile_pool(name="onp", bufs=4) as onp,
            tc.tile_pool(name="actp", bufs=10) as actp,
            tc.tile_pool(name="oop", bufs=3) as oop,
            tc.tile_pool(name="ps_mm", bufs=2, space="PSUM") as ps_mm,
            tc.tile_pool(name="ps_s", bufs=2, space="PSUM") as ps_s,
            tc.tile_pool(name="ps_o", bufs=1, space="PSUM") as ps_o,
            tc.tile_pool(name="dram", bufs=1, space="DRAM") as dram,
        ):
            cc_in = dram.tile([N_CORES, 128, TOK_PER_CORE], BF, name="cc_in")
            cc_out = dram.tile([N_CORES, 128, TOK_PER_CORE], BF, name="cc_out")

            # ---- resident constants ----
            mask_sb = const.tile([128, 896], BF, name="mask_sb")
            nc.sync.dma_start(mask_sb[:], maskd[:])
            ident_sb = const.tile([128, 128], BF, name="ident_sb")
            nc.sync.dma_start(ident_sb[:], identd[:])
            wqk_sb = const.tile([128, 2 * KD, 128], BF, name="wqk_sb")
            nc.sync.dma_start(
                wqk_sb[:],
                wqk.rearrange("h (ko p) m -> p (h ko) m", p=128),
            )
            wv_sb = const.tile([128, KD, 128], BF, name="wv_sb")
            nc.sync.dma_start(wv_sb[:], wv.rearrange("(ko p) m -> p ko m", p=128))
            bqk_sb = const.tile([128, 2], FP, name="bqk_sb")
            nc.sync.dma_start(bqk_sb[:], bqk.rearrange("h p one -> p (h one)"))
            bv_sb = const.tile([128, 1], FP, name="bv_sb")
            nc.sync.dma_start(bv_sb[:], bvb[:])
            for b in range(B):
                # ---- QKV projection for this batch (both heads) ----
                qT = qkv.tile([128, S], BF, name="qT", tag="qT")  # rows: h0 dims 0-63, h1 64-127
                kT = qkv.tile([128, S], BF, name="kT", tag="kT")
                vT = qkv.tile([128, S], BF, name="vT", tag="vT")
                for st in range(4):  # 512-token slabs
                    xts = []
                    for kd in range(KD):
                        xt_t = xtp.tile([128, 512], BF, name="xt_t", tag="xt")
                        nc.sync.dma_start(
                            xt_t[:],
                            xt[kd * 128 : (kd + 1) * 128, b * S + st * 512 : b * S + (st + 1) * 512],
                        )
                        xts.append(xt_t)
                    for h in range(2):
                        ps = ps_mm.tile([128, 512], FP, name="ps_qk", tag="mm")
                        for kd in range(KD):
                            nc.tensor.matmul(
                                ps[:],
                                lhsT=wqk_sb[:, h * KD + kd, :],
                                rhs=xts[kd][:],
                                start=(kd == 0),
                                stop=(kd == KD - 1),
                            )
                        nc.vector.tensor_scalar(
                            qT[h * 64 : h * 64 + 64, st * 512 : (st + 1) * 512],
                            ps[0:64, :],
                            bqk_sb[0:64, h : h + 1],
                            None,
                            AOP.add,
                        )
                        nc.vector.tensor_scalar(
                            kT[h * 64 : h * 64 + 64, st * 512 : (st + 1) * 512],
                            ps[64:128, :],
                            bqk_sb[64:128, h : h + 1],
                            None,
                            AOP.add,
                        )
                    ps = ps_mm.tile([128, 512], FP, name="ps_v", tag="mm")
                    for kd in range(KD):
                        nc.tensor.matmul(
                            ps[:],
                            lhsT=wv_sb[:, kd, :],
                            rhs=xts[kd][:],
                            start=(kd == 0),
                            stop=(kd == KD - 1),
                        )
                    nc.vector.tensor_scalar(
                        vT[:, st * 512 : (st + 1) * 512],
                        ps[:],
                        bv_sb[:, 0:1],
                        None,
                        AOP.add,
                    )

                # ---- vT -> v natural [token, hd] with ones columns ----
                vn_tiles = []
                for kc in range(S // 128):
                    pst = ps_mm.tile([128, 128], BF, name="ps_t", tag="mm")
                    nc.tensor.transpose(pst[:], vT[:, kc * 128 : (kc + 1) * 128], ident_sb[:])
                    vn = vnp.tile([128, 130], BF, name="vn", tag="vn")
                    nc.any.tensor_copy(out=vn[:, 0:64], in_=pst[:, 0:64])
                    nc.any.tensor_copy(out=vn[:, 65:129], in_=pst[:, 64:128])
                    nc.vector.tensor_copy(out=vn[:, 64:65], in_=mask_sb[:, 895:896])
                    nc.vector.tensor_copy(out=vn[:, 129:130], in_=mask_sb[:, 895:896])
                    vn_tiles.append(vn)

                # ---- causal attention, scores transposed [k, q] ----
                for qi in range(4):  # 512-wide query tiles
                    po = [
                        ps_o.tile([65, 512], FP, name=f"po{h}", tag=f"o{h}")
                        for h in range(2)
                    ]
                    nki = 4 * (qi + 1)
                    for ki in range(nki):
                        pss = ps_s.tile([128, 1024], FP, name="ps_sc", tag="sc")
                        for h in range(2):
                            nc.tensor.matmul(
                                pss[:, h * 512 : (h + 1) * 512],
                                lhsT=kT[h * 64 : h * 64 + 64, ki * 128 : (ki + 1) * 128],
                                rhs=qT[h * 64 : h * 64 + 64, qi * 512 : (qi + 1) * 512],
                                start=True,
                                stop=True,
                                tile_position=(h * 64, 0),
                            )
                        es = esp.tile([128, 1024], BF, name="es", tag="es")
                        nc.scalar.activation(es[:], pss[:], AFT.Exp)
                        if ki >= 4 * qi:  # diagonal tile: multiplicative causal mask
                            j = ki - 4 * qi
                            for h in range(2):
                                nc.vector.tensor_tensor(
                                    es[:, h * 512 : (h + 1) * 512],
                                    es[:, h * 512 : (h + 1) * 512],
                                    mask_sb[:, 384 - 128 * j : 896 - 128 * j],
                                    AOP.mult,
                                )
                        for h in range(2):
                            nc.tensor.matmul(
                                po[h][:],
                                lhsT=vn_tiles[ki][:, h * 65 : (h + 1) * 65],
                                rhs=es[:, h * 512 : (h + 1) * 512],
                                start=(ki == 0),
                                stop=(ki == nki - 1),
                            )
                    # normalize and ship to the A2A send buffer
                    j_rank = 2 * b + qi // 2
                    off = (qi % 2) * 512
                    for h in range(2):
                        oc = small.tile([65, 512], FP, name="oc", tag="oc")
                        nc.vector.tensor_copy(out=oc[:], in_=po[h][:])
                        den = small.tile([1, 512], FP, name="den", tag="den")
                        nc.vector.tensor_copy(out=den[:], in_=oc[64:65, :])
                        bc = small.tile([64, 512], FP, name="bc", tag="bc")
                        nc.gpsimd.partition_broadcast(bc[:], den[0:1, :], channels=64)
                        rc = small.tile([64, 512], FP, name="rc", tag="rc")
                        nc.vector.reciprocal_approx_fast(out=rc[:], in_=bc[:])
                        on = onp.tile([64, 512], BF, name="on", tag="on")
                        nc.vector.tensor_tensor(on[:], oc[0:64, :], rc[:], AOP.mult)
                        nc.sync.dma_start(
                            cc_in[j_rank, h * 64 : (h + 1) * 64, off : off + 512],
                            on[:],
                        )

            wo_sb = const.tile([128, KD, D], BF, name="wo_sb")
            nc.sync.dma_start(wo_sb[:], wo.rearrange("(ko p) n -> p ko n", p=128))

            # ---- reshard heads -> tokens ----
            nc.gpsimd.collective_compute(
                "AllToAll",
                AOP.bypass,
                replica_groups=[list(range(N_CORES))],
                ins=[cc_in[:].opt()],
                outs=[cc_out[:].opt()],
            )

            # ---- output projection for my 1024 tokens ----
            for tt in range(TOK_PER_CORE // 128):
                acts = []
                for ft in range(N_CORES):
                    at = actp.tile([128, 128], BF, name="at", tag="at")
                    nc.sync.dma_start(at[:], cc_out[ft, :, tt * 128 : (tt + 1) * 128])
                    acts.append(at)
                for nn in range(2):
                    ps = ps_mm.tile([128, 512], FP, name="ps_op", tag="mm")
                    for ft in range(N_CORES):
                        nc.tensor.matmul(
                            ps[:],
                            lhsT=acts[ft][:],
                            rhs=wo_sb[:, ft, nn * 512 : (nn + 1) * 512],
                            start=(ft == 0),
                            stop=(ft == N_CORES - 1),
                        )
                    oo = oop.tile([128, 512], FP, name="oo", tag="oo")
                    nc.any.tensor_copy(out=oo[:], in_=ps[:])
                    nc.sync.dma_start(
                        out[tt * 128 : (tt + 1) * 128, nn * 512 : (nn + 1) * 512], oo[:]
                    )

    nc.finalize()
    return nc


_NC_CACHE = None


def _get_nc():
    global _NC_CACHE
    if _NC_CACHE is None:
        _NC_CACHE = build_nc()
    return _NC_CACHE


def make_in_maps(x, Wqkv, bqkv, Wo):
    import ml_dtypes

    bf16 = ml_dtypes.bfloat16
    scale = HD ** -0.5
    xtn = np.ascontiguousarray(x.reshape(NT, D).T).astype(bf16)  # [D, NT]
    mask = (np.arange(896)[None, :] - 384 >= np.arange(128)[:, None]).astype(bf16)
    ident = np.eye(128, dtype=np.float32).astype(bf16)
    wo = np.ascontiguousarray(Wo).astype(bf16)
    in_maps = []
    for c in range(N_CORES):
        h0, h1 = 2 * c, 2 * c + 1
        wqk_c = np.stack(
            [
                np.concatenate(
                    [Wqkv[h][:, 0:64] * scale, Wqkv[h][:, 64:128]], axis=1
                )
                for h in (h0, h1)
            ]
        ).astype(bf16)
        wv_c = np.concatenate(
            [Wqkv[h0][:, 128:192], Wqkv[h1][:, 128:192]], axis=1
        ).astype(bf16)
        bqk_c = np.stack(
            [
                np.concatenate([bqkv[h][0:64] * scale, bqkv[h][64:128]])[:, None]
                for h in (h0, h1)
            ]
        ).astype(np.float32)
        bv_c = np.concatenate([bqkv[h0][128:192], bqkv[h1][128:192]])[:, None].astype(
            np.float32
        )
        in_maps.append(
            {
                "xt": xtn,
                "wqk": np.ascontiguousarray(wqk_c),
                "wv": np.ascontiguousarray(wv_c),
                "bqk": np.ascontiguousarray(bqk_c),
                "bv": np.ascontiguousarray(bv_c),
                "wo": wo,
                "mask": mask,
                "ident": ident,
            }
        )
    return in_maps


def run_cores(in_maps, trace=False, trace_kwargs=None):
    nc = _get_nc()
    kwargs = {}
    if trace:
        kwargs["trace"] = True
        if trace_kwargs:
            kwargs["trace_kwargs"] = trace_kwargs
    return bass_utils.run_bass_kernel_spmd(
        nc, in_maps, core_ids=list(range(N_CORES)), **kwargs
    )


def kernel(x, Wqkv, bqkv, Wo, bo):
    x = np.asarray(x, dtype=np.float32)
    Wqkv = np.asarray(Wqkv, dtype=np.float32)
    bqkv = np.asarray(bqkv, dtype=np.float32)
    Wo = np.asarray(Wo, dtype=np.float32)
    bo = np.asarray(bo, dtype=np.float32)

    in_maps = make_in_maps(x, Wqkv, bqkv, Wo)
    res = run_cores(in_maps)
    full = np.concatenate([res.results[c]["out"] for c in range(N_CORES)], axis=0)
    full = full + bo[None, :]
    return full.reshape(B, S, D)
